# revision 1
# baseline (speedup 1.0000x reference)
"""GCNConv message-passing kernel for 8 Trainium2 NeuronCores.

Strategy (edge-parallel, dst-sharded, zero collectives):
- Host: sort edges by dst, split into 8 contiguous dst-node ranges with
  ~equal edge counts (SPMD shards). Within each core, greedily pack
  consecutive dst nodes into "groups" (<=128 nodes each = one PSUM
  window). The node table is split into 4 quarters of <=32767 rows so
  int16 dma_gather indices can address them; each group's edges are
  binned by src-quarter into 4 quarter-pure gather calls.
- Device per group: 4x dma_gather (one per quarter; trailing -1 indices
  make the Q7 descriptor generator skip padding at runtime, which keeps
  the single SPMD program correct for every core's different edge
  counts), build one-hot*weight assignment tiles A[e, dstrel] on the
  vector engine, segment-sum via PE matmuls accumulated in a PSUM
  window, fused with the linear term X@W+b (host ships X^T packed per
  group plus a ones-row so the bias rides the same matmul). Evict each
  window to a staging DRAM tensor at fixed offsets.
- Host: reassemble the full [N, 64] output from per-core stagings.
"""

import numpy as np

from concourse import bass, bacc, mybir
from concourse.tile import TileContext
from concourse.bass_utils import run_bass_kernel_spmd
from concourse.library_config import mlp

N_CORES = 8
_LAST_RUN = {}
P = 128          # partitions / edge-tile size
SPAN = 128       # dst nodes per PSUM window (group)
TQ = 8           # max tiles per (group, quarter); 1024-idx calls are the proven-stable single_packet regime
TILES = 4 * TQ   # tiles per group
CALL_IDXS = TQ * P          # 1280 idx slots per gather call
IDXC = CALL_IDXS // 16      # idx columns per call in the wrapped layout
IDX_PART = 32    # queue-0 gather reads idx stripes from partitions 0-31 only


def _plan_core(src_s, dst_s, w_s, n0, n1, qrows):
    """Greedy-pack nodes [n0, n1) into groups; emit per-group per-quarter
    edge slot arrays. src_s/dst_s/w_s are this core's edges sorted by dst."""
    nnodes = n1 - n0
    deg_q = np.zeros((nnodes, 4), np.int64)
    quarter = src_s // qrows
    drel = dst_s - n0
    np.add.at(deg_q, (drel, quarter), 1)

    cap = TQ * P
    groups = []  # (nstart, nend) absolute node ids
    a = 0
    cq = np.zeros(4, np.int64)
    for n in range(nnodes):
        newcq = cq + deg_q[n]
        if (n - a) >= SPAN or (newcq > cap).any():
            groups.append((n0 + a, n0 + n))
            a = n
            cq = deg_q[n].copy()
        else:
            cq = newcq
    groups.append((n0 + a, n0 + nnodes))
    return groups


def _pack_core(src_s, dst_s, w_s, groups, G, qrows):
    """Build this core's device input slabs for G group slots."""
    idx_slab = np.full((16, G * 4 * IDXC), -1, np.int16)
    cnts = np.zeros((1, G * 4), np.int32)
    w_slab = np.zeros((P, G * TILES), np.float32)
    dr_slab = np.zeros((P, G * TILES), np.float32)
    quarter = src_s // qrows
    srcrel = (src_s - quarter * qrows).astype(np.int64)

    for g, (a, b) in enumerate(groups):
        e0 = np.searchsorted(dst_s, a, "left")
        e1 = np.searchsorted(dst_s, b, "left")
        qv = quarter[e0:e1]
        for q in range(4):
            sel = np.nonzero(qv == q)[0]
            cq = len(sel)
            assert cq <= CALL_IDXS, f"group {g} quarter {q} overflow: {cq}"
            cnts[0, g * 4 + q] = cq
            if cq == 0:
                continue
            ids = srcrel[e0:e1][sel].astype(np.int16)
            # wrapped idx layout: slot j -> (partition j%16, col j//16)
            buf = np.full(CALL_IDXS, -1, np.int16)
            buf[:cq] = ids
            c0 = (g * 4 + q) * IDXC
            idx_slab[:, c0:c0 + IDXC] = buf.reshape(IDXC, 16).T
            # slot j -> (tile q*TQ + j//128, partition j%128)
            tbase = g * TILES + q * TQ
            wv = w_s[e0:e1][sel].astype(np.float32)
            dv = (dst_s[e0:e1][sel] - a).astype(np.float32)
            nt = (cq + P - 1) // P
            wbuf = np.zeros(nt * P, np.float32)
            dbuf = np.zeros(nt * P, np.float32)
            wbuf[:cq] = wv
            dbuf[:cq] = dv
            w_slab[:, tbase:tbase + nt] = wbuf.reshape(nt, P).T
            dr_slab[:, tbase:tbase + nt] = dbuf.reshape(nt, P).T

    idx_rep = np.tile(idx_slab, (IDX_PART // 16, 1))
    return idx_rep, w_slab, dr_slab, cnts


def _build_program(G, qrows, d, trace_label=""):
    """One SPMD program shared by all 8 cores."""
    nc = bacc.Bacc("TRN2", target_bir_lowering=False, debug=False,
                   num_devices=N_CORES)
    f32 = mybir.dt.float32
    tq_t = [nc.dram_tensor(f"tq{q}", [qrows, d], f32, kind="ExternalInput").ap()
            for q in range(4)]
    idx_in = nc.dram_tensor("idxs", [IDX_PART, G * 4 * IDXC], mybir.dt.int16,
                            kind="ExternalInput").ap()
    w_in = nc.dram_tensor("wslab", [P, G * TILES], f32, kind="ExternalInput").ap()
    dr_in = nc.dram_tensor("drslab", [P, G * TILES], f32, kind="ExternalInput").ap()
    xtg_in = nc.dram_tensor("xtg", [d + 1, G * SPAN], f32, kind="ExternalInput").ap()
    waug_in = nc.dram_tensor("waug", [d + 1, d], f32, kind="ExternalInput").ap()
    iota_in = nc.dram_tensor("iota", [P, SPAN], f32, kind="ExternalInput").ap()
    cnt_in = nc.dram_tensor("cnts", [1, G * 4], mybir.dt.int32, kind="ExternalInput").ap()
    stage = nc.dram_tensor("stage", [G * SPAN, d], f32, kind="ExternalOutput").ap()

    with TileContext(nc) as tc:
        with tc.tile_pool(name="res", bufs=1) as res, \
             tc.tile_pool(name="msgp", bufs=3) as msgp, \
             tc.tile_pool(name="ap", bufs=3) as apool, \
             tc.tile_pool(name="evp", bufs=3) as evp, \
             tc.tile_pool(name="pp", bufs=2, space="PSUM") as pp:
            nc.gpsimd.load_library(mlp)
            idx_t = res.tile([IDX_PART, G * 4 * IDXC], mybir.dt.int16)
            nc.sync.dma_start(out=idx_t[:], in_=idx_in[:])
            w_t = res.tile([P, G * TILES], f32)
            nc.sync.dma_start(out=w_t[:], in_=w_in[:])
            dr_t = res.tile([P, G * TILES], f32)
            nc.sync.dma_start(out=dr_t[:], in_=dr_in[:])
            xtg_t = res.tile([d + 1, G * SPAN], f32)
            nc.sync.dma_start(out=xtg_t[:], in_=xtg_in[:])
            waug_t = res.tile([d + 1, d], f32)
            nc.sync.dma_start(out=waug_t[:], in_=waug_in[:])
            iota_t = res.tile([P, SPAN], f32)
            nc.sync.dma_start(out=iota_t[:], in_=iota_in[:])
            cnt_t = res.tile([1, G * 4], mybir.dt.int32)
            nc.sync.dma_start(out=cnt_t[:], in_=cnt_in[:])
            creg = nc.gpsimd.alloc_register("cnt")

            for g in range(G):
                msg = msgp.tile([P, TILES, d], f32)
                if g < 3:
                    nc.vector.memset(msg[:], 0.0)
                for q in range(4):
                    c = g * 4 + q
                    c0 = c * IDXC
                    nc.gpsimd.reg_load(creg, cnt_t[0:1, c:c + 1])
                    nc.gpsimd.dma_gather(
                        out_ap=msg[:, q * TQ:(q + 1) * TQ, :],
                        in_ap=tq_t[q][:],
                        idxs_ap=idx_t[:, c0:c0 + IDXC],
                        num_idxs=CALL_IDXS,
                        num_idxs_reg=creg,
                        elem_size=d,
                        single_packet=True,
                    )
                psum = pp.tile([SPAN, d], f32, space="PSUM")
                nc.tensor.matmul(
                    psum[:],
                    lhsT=xtg_t[:, g * SPAN:(g + 1) * SPAN],
                    rhs=waug_t[:],
                    start=True, stop=False,
                )
                for t in range(TILES):
                    gt = g * TILES + t
                    a_t = apool.tile([P, SPAN], f32)
                    nc.vector.tensor_scalar(
                        out=a_t[:], in0=iota_t[:],
                        scalar1=dr_t[:, gt:gt + 1], scalar2=w_t[:, gt:gt + 1],
                        op0=mybir.AluOpType.is_equal, op1=mybir.AluOpType.mult,
                    )
                    nc.tensor.matmul(
                        psum[:], lhsT=a_t[:], rhs=msg[:, t, :],
                        start=False, stop=(t == TILES - 1),
                    )
                ev = evp.tile([SPAN, d], f32)
                nc.vector.tensor_copy(out=ev[:], in_=psum[:])
                nc.sync.dma_start(out=stage[g * SPAN:(g + 1) * SPAN, :], in_=ev[:])
    nc.compile()
    return nc


def kernel(node_states, edge_weight, W, b, src_index, dst_index):
    node_states = np.asarray(node_states, dtype=np.float32)
    edge_weight = np.asarray(edge_weight, dtype=np.float32)
    W = np.asarray(W, dtype=np.float32)
    b = np.asarray(b, dtype=np.float32)
    src_index = np.asarray(src_index).astype(np.int64)
    dst_index = np.asarray(dst_index).astype(np.int64)

    N, d = node_states.shape
    E = src_index.shape[0]
    qrows = (N + 3) // 4
    assert qrows <= 32767, "int16 quarter index limit"

    order = np.argsort(dst_index, kind="stable")
    src_s = src_index[order]
    dst_s = dst_index[order]
    w_s = edge_weight[order]

    # core boundaries: ~equal edge counts, snapped to node boundaries
    node_bounds = [0]
    for k in range(1, N_CORES):
        nb = int(dst_s[min(k * E // N_CORES, E - 1)])
        node_bounds.append(max(nb, node_bounds[-1]))
    node_bounds.append(N)

    plans = []
    for k in range(N_CORES):
        n0, n1 = node_bounds[k], node_bounds[k + 1]
        e0 = np.searchsorted(dst_s, n0, "left")
        e1 = np.searchsorted(dst_s, n1, "left")
        plans.append((n0, n1, src_s[e0:e1], dst_s[e0:e1], w_s[e0:e1]))

    groups_per_core = [
        _plan_core(ss, ds, ws, n0, n1, qrows)
        for (n0, n1, ss, ds, ws) in plans
    ]
    G = max(len(g) for g in groups_per_core)

    # device program
    nc = _build_program(G, qrows, d)

    # table quarters (shared across cores)
    tpad = np.zeros((qrows * 4, d), np.float32)
    tpad[:N] = node_states
    quarters = {f"tq{q}": tpad[q * qrows:(q + 1) * qrows] for q in range(4)}
    waug = np.concatenate([W, b[None, :]], axis=0).astype(np.float32)
    iota = np.broadcast_to(np.arange(SPAN, dtype=np.float32), (P, SPAN)).copy()

    in_maps = []
    for k in range(N_CORES):
        n0, n1, ss, ds, ws = plans[k]
        groups = groups_per_core[k]
        idx_rep, w_slab, dr_slab, cnts = _pack_core(ss, ds, ws, groups, G, qrows)
        xtg = np.zeros((d + 1, G * SPAN), np.float32)
        for g, (a, bb) in enumerate(groups):
            xtg[:d, g * SPAN:g * SPAN + (bb - a)] = node_states[a:bb].T
            xtg[d, g * SPAN:g * SPAN + (bb - a)] = 1.0
        in_maps.append({
            **quarters,
            "idxs": idx_rep, "wslab": w_slab, "drslab": dr_slab,
            "xtg": xtg, "waug": waug, "iota": iota, "cnts": cnts,
        })

    _LAST_RUN["nc"] = nc
    _LAST_RUN["in_maps"] = in_maps
    res = run_bass_kernel_spmd(nc, in_maps, list(range(N_CORES)))

    out = np.zeros((N, d), np.float32)
    for k in range(N_CORES):
        stage = res.results[k]["stage"]
        for g, (a, bb) in enumerate(groups_per_core[k]):
            out[a:bb] = stage[g * SPAN:g * SPAN + (bb - a)]
    return out



# revision 2
# speedup vs baseline: 1.7712x; 1.7712x over previous
"""GCNConv message-passing kernel for 8 Trainium2 NeuronCores.

Strategy (edge-parallel, dst-sharded, zero collectives):
- Host: sort edges by dst, split into 8 contiguous dst-node ranges with
  ~equal edge counts (SPMD shards). Within each core, greedily pack
  consecutive dst nodes into "groups" (<=128 nodes each = one PSUM
  window). The node table is split into 4 quarters of <=32767 rows so
  int16 dma_gather indices can address them; each group's edges are
  binned by src-quarter into 4 quarter-pure gather calls.
- Device per group: 4x dma_gather (one per quarter; trailing -1 indices
  make the Q7 descriptor generator skip padding at runtime, which keeps
  the single SPMD program correct for every core's different edge
  counts), build one-hot*weight assignment tiles A[e, dstrel] on the
  vector engine, segment-sum via PE matmuls accumulated in a PSUM
  window, fused with the linear term X@W+b (host ships X^T packed per
  group plus a ones-row so the bias rides the same matmul). Evict each
  window to a staging DRAM tensor at fixed offsets.
- Host: reassemble the full [N, 64] output from per-core stagings.
"""

import numpy as np

from concourse import bass, bacc, mybir
from concourse.tile import TileContext
from concourse.bass_utils import run_bass_kernel_spmd
from concourse.library_config import mlp

N_CORES = 8
_LAST_RUN = {}
P = 128          # partitions / edge-tile size
SPAN = 128       # dst nodes per PSUM window (group)
TQ = 8           # max tiles per (group, quarter); 1024-idx calls are the proven-stable single_packet regime
TILES = 4 * TQ   # tiles per group
CALL_IDXS = TQ * P          # 1280 idx slots per gather call
IDXC = CALL_IDXS // 16      # idx columns per call in the wrapped layout
IDX_PART = 128   # 4-queue gathers: queue q's Q7 core pair reads idx from partitions 32q..32q+31


def _plan_core(src_s, dst_s, w_s, n0, n1, qrows):
    """Greedy-pack nodes [n0, n1) into groups; emit per-group per-quarter
    edge slot arrays. src_s/dst_s/w_s are this core's edges sorted by dst."""
    nnodes = n1 - n0
    deg_q = np.zeros((nnodes, 4), np.int64)
    quarter = src_s // qrows
    drel = dst_s - n0
    np.add.at(deg_q, (drel, quarter), 1)

    cap = TQ * P
    groups = []  # (nstart, nend) absolute node ids
    a = 0
    cq = np.zeros(4, np.int64)
    for n in range(nnodes):
        newcq = cq + deg_q[n]
        if (n - a) >= SPAN or (newcq > cap).any():
            groups.append((n0 + a, n0 + n))
            a = n
            cq = deg_q[n].copy()
        else:
            cq = newcq
    groups.append((n0 + a, n0 + nnodes))
    return groups


def _pack_core(src_s, dst_s, w_s, groups, G, qrows):
    """Build this core's device input slabs for G group slots."""
    idx_slab = np.full((16, G * 4 * IDXC), -1, np.int16)
    cnts = np.zeros((1, G * 4), np.int32)
    w_slab = np.zeros((P, G * TILES), np.float32)
    dr_slab = np.zeros((P, G * TILES), np.float32)
    quarter = src_s // qrows
    srcrel = (src_s - quarter * qrows).astype(np.int64)

    for g, (a, b) in enumerate(groups):
        e0 = np.searchsorted(dst_s, a, "left")
        e1 = np.searchsorted(dst_s, b, "left")
        qv = quarter[e0:e1]
        for q in range(4):
            sel = np.nonzero(qv == q)[0]
            cq = len(sel)
            assert cq <= CALL_IDXS, f"group {g} quarter {q} overflow: {cq}"
            cnts[0, g * 4 + q] = cq
            if cq == 0:
                continue
            ids = srcrel[e0:e1][sel].astype(np.int16)
            # wrapped idx layout: slot j -> (partition j%16, col j//16)
            buf = np.full(CALL_IDXS, -1, np.int16)
            buf[:cq] = ids
            c0 = (g * 4 + q) * IDXC
            idx_slab[:, c0:c0 + IDXC] = buf.reshape(IDXC, 16).T
            # slot j -> (tile q*TQ + j//128, partition j%128)
            tbase = g * TILES + q * TQ
            wv = w_s[e0:e1][sel].astype(np.float32)
            dv = (dst_s[e0:e1][sel] - a).astype(np.float32)
            nt = (cq + P - 1) // P
            wbuf = np.zeros(nt * P, np.float32)
            dbuf = np.zeros(nt * P, np.float32)
            wbuf[:cq] = wv
            dbuf[:cq] = dv
            w_slab[:, tbase:tbase + nt] = wbuf.reshape(nt, P).T
            dr_slab[:, tbase:tbase + nt] = dbuf.reshape(nt, P).T

    idx_rep = np.tile(idx_slab, (IDX_PART // 16, 1))
    return idx_rep, w_slab, dr_slab, cnts


def _build_program(G, qrows, d, trace_label=""):
    """One SPMD program shared by all 8 cores."""
    nc = bacc.Bacc("TRN2", target_bir_lowering=False, debug=False,
                   num_devices=N_CORES, num_swdge_queues=4)
    f32 = mybir.dt.float32
    tq_t = [nc.dram_tensor(f"tq{q}", [qrows, d], f32, kind="ExternalInput").ap()
            for q in range(4)]
    idx_in = nc.dram_tensor("idxs", [IDX_PART, G * 4 * IDXC], mybir.dt.int16,
                            kind="ExternalInput").ap()
    w_in = nc.dram_tensor("wslab", [P, G * TILES], f32, kind="ExternalInput").ap()
    dr_in = nc.dram_tensor("drslab", [P, G * TILES], f32, kind="ExternalInput").ap()
    xtg_in = nc.dram_tensor("xtg", [d + 1, G * SPAN], f32, kind="ExternalInput").ap()
    waug_in = nc.dram_tensor("waug", [d + 1, d], f32, kind="ExternalInput").ap()
    iota_in = nc.dram_tensor("iota", [P, SPAN], f32, kind="ExternalInput").ap()
    cnt_in = nc.dram_tensor("cnts", [1, G * 4], mybir.dt.int32, kind="ExternalInput").ap()
    stage = nc.dram_tensor("stage", [G * SPAN, d], f32, kind="ExternalOutput").ap()

    with TileContext(nc) as tc:
        with tc.tile_pool(name="res", bufs=1) as res, \
             tc.tile_pool(name="msgp", bufs=6) as msgp, \
             tc.tile_pool(name="ap", bufs=3) as apool, \
             tc.tile_pool(name="evp", bufs=3) as evp, \
             tc.tile_pool(name="pp", bufs=2, space="PSUM") as pp:
            nc.gpsimd.load_library(mlp)
            idx_t = res.tile([IDX_PART, G * 4 * IDXC], mybir.dt.int16)
            nc.sync.dma_start(out=idx_t[:], in_=idx_in[:])
            w_t = res.tile([P, G * TILES], f32)
            nc.sync.dma_start(out=w_t[:], in_=w_in[:])
            dr_t = res.tile([P, G * TILES], f32)
            nc.sync.dma_start(out=dr_t[:], in_=dr_in[:])
            xtg_t = res.tile([d + 1, G * SPAN], f32)
            nc.sync.dma_start(out=xtg_t[:], in_=xtg_in[:])
            waug_t = res.tile([d + 1, d], f32)
            nc.sync.dma_start(out=waug_t[:], in_=waug_in[:])
            iota_t = res.tile([P, SPAN], f32)
            nc.sync.dma_start(out=iota_t[:], in_=iota_in[:])
            cnt_t = res.tile([1, G * 4], mybir.dt.int32)
            nc.sync.dma_start(out=cnt_t[:], in_=cnt_in[:])
            creg = nc.gpsimd.alloc_register("cnt")

            for g in range(G):
                msg = msgp.tile([P, TILES, d], f32)
                if g < 6:
                    nc.vector.memset(msg[:], 0.0)
                for q in range(4):
                    c = g * 4 + q
                    c0 = c * IDXC
                    nc.gpsimd.reg_load(creg, cnt_t[0:1, c:c + 1])
                    nc.gpsimd.dma_gather(
                        out_ap=msg[:, q * TQ:(q + 1) * TQ, :],
                        in_ap=tq_t[q][:],
                        idxs_ap=idx_t[:, c0:c0 + IDXC],
                        num_idxs=CALL_IDXS,
                        num_idxs_reg=creg,
                        elem_size=d,
                        single_packet=False,
                        queue_num=q,
                    )
                psum = pp.tile([SPAN, d], f32, space="PSUM")
                nc.tensor.matmul(
                    psum[:],
                    lhsT=xtg_t[:, g * SPAN:(g + 1) * SPAN],
                    rhs=waug_t[:],
                    start=True, stop=False,
                )
                for t in range(TILES):
                    gt = g * TILES + t
                    a_t = apool.tile([P, SPAN], f32)
                    nc.vector.tensor_scalar(
                        out=a_t[:], in0=iota_t[:],
                        scalar1=dr_t[:, gt:gt + 1], scalar2=w_t[:, gt:gt + 1],
                        op0=mybir.AluOpType.is_equal, op1=mybir.AluOpType.mult,
                    )
                    nc.tensor.matmul(
                        psum[:], lhsT=a_t[:], rhs=msg[:, t, :],
                        start=False, stop=(t == TILES - 1),
                    )
                ev = evp.tile([SPAN, d], f32)
                nc.vector.tensor_copy(out=ev[:], in_=psum[:])
                nc.sync.dma_start(out=stage[g * SPAN:(g + 1) * SPAN, :], in_=ev[:])
    nc.compile()
    return nc


def kernel(node_states, edge_weight, W, b, src_index, dst_index):
    node_states = np.asarray(node_states, dtype=np.float32)
    edge_weight = np.asarray(edge_weight, dtype=np.float32)
    W = np.asarray(W, dtype=np.float32)
    b = np.asarray(b, dtype=np.float32)
    src_index = np.asarray(src_index).astype(np.int64)
    dst_index = np.asarray(dst_index).astype(np.int64)

    N, d = node_states.shape
    E = src_index.shape[0]
    qrows = (N + 3) // 4
    assert qrows <= 32767, "int16 quarter index limit"

    order = np.argsort(dst_index, kind="stable")
    src_s = src_index[order]
    dst_s = dst_index[order]
    w_s = edge_weight[order]

    # core boundaries: ~equal edge counts, snapped to node boundaries
    node_bounds = [0]
    for k in range(1, N_CORES):
        nb = int(dst_s[min(k * E // N_CORES, E - 1)])
        node_bounds.append(max(nb, node_bounds[-1]))
    node_bounds.append(N)

    plans = []
    for k in range(N_CORES):
        n0, n1 = node_bounds[k], node_bounds[k + 1]
        e0 = np.searchsorted(dst_s, n0, "left")
        e1 = np.searchsorted(dst_s, n1, "left")
        plans.append((n0, n1, src_s[e0:e1], dst_s[e0:e1], w_s[e0:e1]))

    groups_per_core = [
        _plan_core(ss, ds, ws, n0, n1, qrows)
        for (n0, n1, ss, ds, ws) in plans
    ]
    G = max(len(g) for g in groups_per_core)

    # device program
    nc = _build_program(G, qrows, d)

    # table quarters (shared across cores)
    tpad = np.zeros((qrows * 4, d), np.float32)
    tpad[:N] = node_states
    quarters = {f"tq{q}": tpad[q * qrows:(q + 1) * qrows] for q in range(4)}
    waug = np.concatenate([W, b[None, :]], axis=0).astype(np.float32)
    iota = np.broadcast_to(np.arange(SPAN, dtype=np.float32), (P, SPAN)).copy()

    in_maps = []
    for k in range(N_CORES):
        n0, n1, ss, ds, ws = plans[k]
        groups = groups_per_core[k]
        idx_rep, w_slab, dr_slab, cnts = _pack_core(ss, ds, ws, groups, G, qrows)
        xtg = np.zeros((d + 1, G * SPAN), np.float32)
        for g, (a, bb) in enumerate(groups):
            xtg[:d, g * SPAN:g * SPAN + (bb - a)] = node_states[a:bb].T
            xtg[d, g * SPAN:g * SPAN + (bb - a)] = 1.0
        in_maps.append({
            **quarters,
            "idxs": idx_rep, "wslab": w_slab, "drslab": dr_slab,
            "xtg": xtg, "waug": waug, "iota": iota, "cnts": cnts,
        })

    _LAST_RUN["nc"] = nc
    _LAST_RUN["in_maps"] = in_maps
    res = run_bass_kernel_spmd(nc, in_maps, list(range(N_CORES)))

    out = np.zeros((N, d), np.float32)
    for k in range(N_CORES):
        stage = res.results[k]["stage"]
        for g, (a, bb) in enumerate(groups_per_core[k]):
            out[a:bb] = stage[g * SPAN:g * SPAN + (bb - a)]
    return out



# revision 3
# speedup vs baseline: 2.2494x; 1.2700x over previous
"""GCNConv message-passing kernel for 8 Trainium2 NeuronCores.

Strategy (edge-parallel, dst-sharded, zero collectives):
- Host: sort edges by dst, split into 8 contiguous dst-node ranges with
  ~equal edge counts (SPMD shards). Within each core, greedily pack
  consecutive dst nodes into "groups" (<=128 nodes each = one PSUM
  window). The node table is split into 4 quarters of <=32767 rows so
  int16 dma_gather indices can address them; each group's edges are
  binned by src-quarter into 4 quarter-pure gather calls.
- Device per group: 4x dma_gather (one per quarter; trailing -1 indices
  make the Q7 descriptor generator skip padding at runtime, which keeps
  the single SPMD program correct for every core's different edge
  counts), build one-hot*weight assignment tiles A[e, dstrel] on the
  vector engine, segment-sum via PE matmuls accumulated in a PSUM
  window, fused with the linear term X@W+b (host ships X^T packed per
  group plus a ones-row so the bias rides the same matmul). Evict each
  window to a staging DRAM tensor at fixed offsets.
- Host: reassemble the full [N, 64] output from per-core stagings.
"""

import ml_dtypes
import numpy as np

BF16 = ml_dtypes.bfloat16

from concourse import bass, bacc, mybir
from concourse.tile import TileContext
from concourse.bass_utils import run_bass_kernel_spmd
from concourse.library_config import mlp

N_CORES = 8
_LAST_RUN = {}
P = 128          # partitions / edge-tile size
SPAN = 128       # dst nodes per PSUM window (group)
TQ = 8           # max tiles per (group, quarter); 1024-idx calls are the proven-stable single_packet regime
TILES = 4 * TQ   # tiles per group
CALL_IDXS = TQ * P          # 1280 idx slots per gather call
IDXC = CALL_IDXS // 16      # idx columns per call in the wrapped layout
IDX_PART = 128   # 4-queue gathers: queue q's Q7 core pair reads idx from partitions 32q..32q+31


def _plan_core(src_s, dst_s, w_s, n0, n1, qrows):
    """Greedy-pack nodes [n0, n1) into groups; emit per-group per-quarter
    edge slot arrays. src_s/dst_s/w_s are this core's edges sorted by dst."""
    nnodes = n1 - n0
    deg_q = np.zeros((nnodes, 4), np.int64)
    quarter = src_s // qrows
    drel = dst_s - n0
    np.add.at(deg_q, (drel, quarter), 1)

    cap = TQ * P
    groups = []  # (nstart, nend) absolute node ids
    a = 0
    cq = np.zeros(4, np.int64)
    for n in range(nnodes):
        newcq = cq + deg_q[n]
        if (n - a) >= SPAN or (newcq > cap).any():
            groups.append((n0 + a, n0 + n))
            a = n
            cq = deg_q[n].copy()
        else:
            cq = newcq
    groups.append((n0 + a, n0 + nnodes))
    return groups


def _pack_core(src_s, dst_s, w_s, groups, G, qrows):
    """Build this core's device input slabs for G group slots."""
    idx_slab = np.full((16, G * 4 * IDXC), -1, np.int16)
    cnts = np.zeros((1, G * 4), np.int32)
    w_slab = np.zeros((P, G * TILES), np.float32)
    dr_slab = np.zeros((P, G * TILES), np.float32)
    quarter = src_s // qrows
    srcrel = (src_s - quarter * qrows).astype(np.int64)

    for g, (a, b) in enumerate(groups):
        e0 = np.searchsorted(dst_s, a, "left")
        e1 = np.searchsorted(dst_s, b, "left")
        qv = quarter[e0:e1]
        for q in range(4):
            sel = np.nonzero(qv == q)[0]
            cq = len(sel)
            assert cq <= CALL_IDXS, f"group {g} quarter {q} overflow: {cq}"
            cnts[0, g * 4 + q] = cq
            if cq == 0:
                continue
            ids = srcrel[e0:e1][sel].astype(np.int16)
            # wrapped idx layout: slot j -> (partition j%16, col j//16)
            buf = np.full(CALL_IDXS, -1, np.int16)
            buf[:cq] = ids
            c0 = (g * 4 + q) * IDXC
            idx_slab[:, c0:c0 + IDXC] = buf.reshape(IDXC, 16).T
            # slot j -> (tile q*TQ + j//128, partition j%128)
            tbase = g * TILES + q * TQ
            wv = w_s[e0:e1][sel].astype(np.float32)
            dv = (dst_s[e0:e1][sel] - a).astype(np.float32)
            nt = (cq + P - 1) // P
            wbuf = np.zeros(nt * P, np.float32)
            dbuf = np.zeros(nt * P, np.float32)
            wbuf[:cq] = wv
            dbuf[:cq] = dv
            w_slab[:, tbase:tbase + nt] = wbuf.reshape(nt, P).T
            dr_slab[:, tbase:tbase + nt] = dbuf.reshape(nt, P).T

    idx_rep = np.tile(idx_slab, (IDX_PART // 16, 1))
    return idx_rep, w_slab, dr_slab, cnts


def _build_program(G, qrows, d, trace_label=""):
    """One SPMD program shared by all 8 cores."""
    nc = bacc.Bacc("TRN2", target_bir_lowering=False, debug=False,
                   num_devices=N_CORES, num_swdge_queues=4)
    f32 = mybir.dt.float32
    bf16 = mybir.dt.bfloat16
    de = 2 * d  # bf16 rows padded to 256B so gather stride stays 256B-aligned
    tq_t = [nc.dram_tensor(f"tq{q}", [qrows, de], bf16, kind="ExternalInput").ap()
            for q in range(4)]
    idx_in = nc.dram_tensor("idxs", [IDX_PART, G * 4 * IDXC], mybir.dt.int16,
                            kind="ExternalInput").ap()
    w_in = nc.dram_tensor("wslab", [P, G * TILES], f32, kind="ExternalInput").ap()
    dr_in = nc.dram_tensor("drslab", [P, G * TILES], f32, kind="ExternalInput").ap()
    xtg_in = nc.dram_tensor("xtg", [d + 1, G * SPAN], bf16, kind="ExternalInput").ap()
    waug_in = nc.dram_tensor("waug", [d + 1, d], bf16, kind="ExternalInput").ap()
    iota_in = nc.dram_tensor("iota", [P, SPAN], bf16, kind="ExternalInput").ap()
    cnt_in = nc.dram_tensor("cnts", [1, G * 4], mybir.dt.int32, kind="ExternalInput").ap()
    stage = nc.dram_tensor("stage", [G * SPAN, d], f32, kind="ExternalOutput").ap()

    with TileContext(nc) as tc:
        with tc.tile_pool(name="res", bufs=1) as res, \
             tc.tile_pool(name="msgp", bufs=6) as msgp, \
             tc.tile_pool(name="ap", bufs=3) as apool, \
             tc.tile_pool(name="evp", bufs=3) as evp, \
             tc.tile_pool(name="pp", bufs=2, space="PSUM") as pp:
            nc.gpsimd.load_library(mlp)
            idx_t = res.tile([IDX_PART, G * 4 * IDXC], mybir.dt.int16)
            nc.sync.dma_start(out=idx_t[:], in_=idx_in[:])
            w_t = res.tile([P, G * TILES], f32)
            nc.sync.dma_start(out=w_t[:], in_=w_in[:])
            dr_t = res.tile([P, G * TILES], f32)
            nc.sync.dma_start(out=dr_t[:], in_=dr_in[:])
            xtg_t = res.tile([d + 1, G * SPAN], bf16)
            nc.sync.dma_start(out=xtg_t[:], in_=xtg_in[:])
            waug_t = res.tile([d + 1, d], bf16)
            nc.sync.dma_start(out=waug_t[:], in_=waug_in[:])
            iota_t = res.tile([P, SPAN], bf16)
            nc.sync.dma_start(out=iota_t[:], in_=iota_in[:])
            cnt_t = res.tile([1, G * 4], mybir.dt.int32)
            nc.sync.dma_start(out=cnt_t[:], in_=cnt_in[:])
            creg = nc.gpsimd.alloc_register("cnt")

            for g in range(G):
                msg = msgp.tile([P, TILES, de], bf16)
                if g < 6:
                    nc.vector.memset(msg[:], 0.0)
                for q in range(4):
                    c = g * 4 + q
                    c0 = c * IDXC
                    nc.gpsimd.reg_load(creg, cnt_t[0:1, c:c + 1])
                    nc.gpsimd.dma_gather(
                        out_ap=msg[:, q * TQ:(q + 1) * TQ, :],
                        in_ap=tq_t[q][:],
                        idxs_ap=idx_t[:, c0:c0 + IDXC],
                        num_idxs=CALL_IDXS,
                        num_idxs_reg=creg,
                        elem_size=de,
                        single_packet=False,
                        queue_num=q,
                    )
                psum = pp.tile([SPAN, d], f32, space="PSUM")
                nc.tensor.matmul(
                    psum[:],
                    lhsT=xtg_t[:, g * SPAN:(g + 1) * SPAN],
                    rhs=waug_t[:],
                    start=True, stop=False,
                )
                for t in range(TILES):
                    gt = g * TILES + t
                    a_t = apool.tile([P, SPAN], bf16)
                    nc.vector.tensor_scalar(
                        out=a_t[:], in0=iota_t[:],
                        scalar1=dr_t[:, gt:gt + 1], scalar2=w_t[:, gt:gt + 1],
                        op0=mybir.AluOpType.is_equal, op1=mybir.AluOpType.mult,
                    )
                    nc.tensor.matmul(
                        psum[:], lhsT=a_t[:], rhs=msg[:, t, 0:d],
                        start=False, stop=(t == TILES - 1),
                    )
                ev = evp.tile([SPAN, d], f32)
                nc.vector.tensor_copy(out=ev[:], in_=psum[:])
                nc.sync.dma_start(out=stage[g * SPAN:(g + 1) * SPAN, :], in_=ev[:])
    nc.compile()
    return nc


def kernel(node_states, edge_weight, W, b, src_index, dst_index):
    node_states = np.asarray(node_states, dtype=np.float32)
    edge_weight = np.asarray(edge_weight, dtype=np.float32)
    W = np.asarray(W, dtype=np.float32)
    b = np.asarray(b, dtype=np.float32)
    src_index = np.asarray(src_index).astype(np.int64)
    dst_index = np.asarray(dst_index).astype(np.int64)

    N, d = node_states.shape
    E = src_index.shape[0]
    qrows = (N + 3) // 4
    assert qrows <= 32767, "int16 quarter index limit"

    order = np.argsort(dst_index, kind="stable")
    src_s = src_index[order]
    dst_s = dst_index[order]
    w_s = edge_weight[order]

    # core boundaries: ~equal edge counts, snapped to node boundaries
    node_bounds = [0]
    for k in range(1, N_CORES):
        nb = int(dst_s[min(k * E // N_CORES, E - 1)])
        node_bounds.append(max(nb, node_bounds[-1]))
    node_bounds.append(N)

    plans = []
    for k in range(N_CORES):
        n0, n1 = node_bounds[k], node_bounds[k + 1]
        e0 = np.searchsorted(dst_s, n0, "left")
        e1 = np.searchsorted(dst_s, n1, "left")
        plans.append((n0, n1, src_s[e0:e1], dst_s[e0:e1], w_s[e0:e1]))

    groups_per_core = [
        _plan_core(ss, ds, ws, n0, n1, qrows)
        for (n0, n1, ss, ds, ws) in plans
    ]
    G = max(len(g) for g in groups_per_core)

    # device program
    nc = _build_program(G, qrows, d)

    # table quarters (shared across cores), bf16 rows padded to 256B
    tpad = np.zeros((qrows * 4, 2 * d), BF16)
    tpad[:N, :d] = node_states.astype(BF16)
    quarters = {f"tq{q}": tpad[q * qrows:(q + 1) * qrows] for q in range(4)}
    waug = np.concatenate([W, b[None, :]], axis=0).astype(BF16)
    iota = np.broadcast_to(np.arange(SPAN).astype(BF16), (P, SPAN)).copy()

    in_maps = []
    for k in range(N_CORES):
        n0, n1, ss, ds, ws = plans[k]
        groups = groups_per_core[k]
        idx_rep, w_slab, dr_slab, cnts = _pack_core(ss, ds, ws, groups, G, qrows)
        xtg = np.zeros((d + 1, G * SPAN), BF16)
        for g, (a, bb) in enumerate(groups):
            xtg[:d, g * SPAN:g * SPAN + (bb - a)] = node_states[a:bb].T.astype(BF16)
            xtg[d, g * SPAN:g * SPAN + (bb - a)] = 1.0
        in_maps.append({
            **quarters,
            "idxs": idx_rep, "wslab": w_slab, "drslab": dr_slab,
            "xtg": xtg, "waug": waug, "iota": iota, "cnts": cnts,
        })

    _LAST_RUN["nc"] = nc
    _LAST_RUN["in_maps"] = in_maps
    res = run_bass_kernel_spmd(nc, in_maps, list(range(N_CORES)))

    out = np.zeros((N, d), np.float32)
    for k in range(N_CORES):
        stage = res.results[k]["stage"]
        for g, (a, bb) in enumerate(groups_per_core[k]):
            out[a:bb] = stage[g * SPAN:g * SPAN + (bb - a)]
    return out



# revision 5
# speedup vs baseline: 4.3994x; 1.9559x over previous
"""GCNConv message-passing kernel for 8 Trainium2 NeuronCores.

Strategy (edge-parallel, dst-sharded, zero collectives):
- Host: sort edges by dst, split into 8 contiguous dst-node ranges with
  ~equal edge counts (SPMD shards). Within each core, greedily pack
  consecutive dst nodes into "groups" (<=128 nodes each = one PSUM
  window). The node table is split into 4 quarters of <=32767 rows so
  int16 dma_gather indices can address them; each group's edges are
  binned by src-quarter into 4 quarter-pure gather calls.
- Gather: node rows stored as bf16 padded to 256B. The 4 quarter-calls
  of each group go to the 4 SWDGE queues (queue q = Q7 core pair q ->
  4x parallel descriptor generation) with single_packet=False so the
  SDMA engines interleave the 4 rings per-descriptor (4 outstanding
  HBM reads per engine). Deep msg-tile pool (8 bufs) keeps many calls
  in flight. Measured ~2.2-2.4 ns/idx vs 8.8 ns/idx single-queue.
- Segment-sum: host precomputes the one-hot*weight assignment tiles
  A[slot, dst_rel] in bf16 and ships them; the device streams them per
  group via HWDGE (no DVE build, no GPSIMD/DVE port contention). PE
  accumulates the 32 per-tile matmuls plus the fused linear term
  X@W+b (host ships X^T per group with a ones-row for the bias) into
  one PSUM window per group, evicted to a staging DRAM tensor.
- Host: reassemble the full [N, 64] output from per-core stagings.
"""

import ml_dtypes
import numpy as np

BF16 = ml_dtypes.bfloat16

from concourse import bacc, mybir
from concourse.tile import TileContext
from concourse.bass_utils import run_bass_kernel_spmd
from concourse.library_config import mlp

N_CORES = 8
_LAST_RUN = {}
P = 128          # partitions / edge-tile size
SPAN = 128       # dst nodes per PSUM window (group)
TQ = 8           # max tiles per (group, quarter); 1024-idx calls are the stable regime
TILES = 4 * TQ   # tiles per group
CALL_IDXS = TQ * P          # 1024 idx slots per gather call
IDXC = CALL_IDXS // 16      # idx columns per call in the wrapped layout
IDX_PART = 128   # idx replicated to all partitions: queue q reads 32q..32q+31
MSG_BUFS = 8


def _plan_core(src_s, dst_s, w_s, n0, n1, qrows):
    """Greedy-pack nodes [n0, n1) into groups; emit per-group per-quarter
    edge slot arrays. src_s/dst_s/w_s are this core's edges sorted by dst."""
    nnodes = n1 - n0
    deg_q = np.zeros((nnodes, 4), np.int64)
    quarter = src_s // qrows
    drel = dst_s - n0
    np.add.at(deg_q, (drel, quarter), 1)

    cap = TQ * P
    groups = []  # (nstart, nend) absolute node ids
    a = 0
    cq = np.zeros(4, np.int64)
    for n in range(nnodes):
        newcq = cq + deg_q[n]
        if (n - a) >= SPAN or (newcq > cap).any():
            groups.append((n0 + a, n0 + n))
            a = n
            cq = deg_q[n].copy()
        else:
            cq = newcq
    groups.append((n0 + a, n0 + nnodes))
    return groups


def _pack_core(src_s, dst_s, w_s, groups, G, qrows):
    """Build this core's device input slabs for G group slots."""
    idx_slab = np.full((128, G * IDXC), -1, np.int16)
    cnts = np.zeros((1, G * 4), np.int32)
    aslab = np.zeros((P, G * TILES * SPAN), np.float32)
    quarter = src_s // qrows
    srcrel = (src_s - quarter * qrows).astype(np.int64)

    for g, (a, b) in enumerate(groups):
        e0 = np.searchsorted(dst_s, a, "left")
        e1 = np.searchsorted(dst_s, b, "left")
        qv = quarter[e0:e1]
        for q in range(4):
            sel = np.nonzero(qv == q)[0]
            cq = len(sel)
            assert cq <= CALL_IDXS, f"group {g} quarter {q} overflow: {cq}"
            cnts[0, g * 4 + q] = cq
            if cq == 0:
                continue
            ids = srcrel[e0:e1][sel].astype(np.int16)
            # wrapped idx layout: slot j -> (partition j%16, col j//16);
            # queue q's Q7 pair reads partitions 32q..32q+31 only
            buf = np.full(CALL_IDXS, -1, np.int16)
            buf[:cq] = ids
            wrap = buf.reshape(IDXC, 16).T
            c0 = g * IDXC
            idx_slab[32 * q:32 * q + 16, c0:c0 + IDXC] = wrap
            idx_slab[32 * q + 16:32 * q + 32, c0:c0 + IDXC] = wrap
            # A tiles: slot j of quarter q -> tile q*TQ + j//128, row j%128
            wv = w_s[e0:e1][sel].astype(np.float32)
            dv = (dst_s[e0:e1][sel] - a).astype(np.int64)
            slots = np.arange(cq)
            tile_idx = g * TILES + q * TQ + slots // P
            aslab[slots % P, tile_idx * SPAN + dv] = wv

    return idx_slab, aslab.astype(BF16), cnts


def _build_program(G, qrows, d):
    """One SPMD program shared by all 8 cores."""
    nc = bacc.Bacc("TRN2", target_bir_lowering=False, debug=False,
                   num_devices=N_CORES, num_swdge_queues=4)
    f32 = mybir.dt.float32
    bf16 = mybir.dt.bfloat16
    de = 2 * d  # bf16 node rows padded to 256B (gather stride constraint)
    tq_t = [nc.dram_tensor(f"tq{q}", [qrows, de], bf16, kind="ExternalInput").ap()
            for q in range(4)]
    idx_in = nc.dram_tensor("idxs", [IDX_PART, G * IDXC], mybir.dt.int16,
                            kind="ExternalInput").ap()
    aslab_in = nc.dram_tensor("aslab", [P, G * TILES * SPAN], bf16,
                              kind="ExternalInput").ap()
    xtg_in = nc.dram_tensor("xtg", [d + 1, G * SPAN], bf16, kind="ExternalInput").ap()
    waug_in = nc.dram_tensor("waug", [d + 1, d], bf16, kind="ExternalInput").ap()
    cnt_in = nc.dram_tensor("cnts", [1, G * 4], mybir.dt.int32, kind="ExternalInput").ap()
    stage = nc.dram_tensor("stage", [G * SPAN, d], f32, kind="ExternalOutput").ap()

    with TileContext(nc) as tc:
        with tc.tile_pool(name="res", bufs=1) as res, \
             tc.tile_pool(name="msgp", bufs=MSG_BUFS) as msgp, \
             tc.tile_pool(name="ag", bufs=3) as agpool, \
             tc.tile_pool(name="evp", bufs=4) as evp, \
             tc.tile_pool(name="pp", bufs=4, space="PSUM") as pp:
            nc.gpsimd.load_library(mlp)
            idx_t = res.tile([IDX_PART, G * IDXC], mybir.dt.int16)
            nc.sync.dma_start(out=idx_t[:], in_=idx_in[:])
            xtg_t = res.tile([d + 1, G * SPAN], bf16)
            nc.sync.dma_start(out=xtg_t[:], in_=xtg_in[:])
            waug_t = res.tile([d + 1, d], bf16)
            nc.sync.dma_start(out=waug_t[:], in_=waug_in[:])
            cnt_t = res.tile([1, G * 4], mybir.dt.int32)
            nc.sync.dma_start(out=cnt_t[:], in_=cnt_in[:])
            creg = nc.gpsimd.alloc_register("cnt")

            for g in range(G):
                msg = msgp.tile([P, TILES, de], bf16)
                if g < MSG_BUFS:
                    nc.vector.memset(msg[:], 0.0)
                for q in range(4):
                    c = g * 4 + q
                    c0 = g * IDXC
                    nc.gpsimd.reg_load(creg, cnt_t[0:1, c:c + 1])
                    nc.gpsimd.dma_gather(
                        out_ap=msg[:, q * TQ:(q + 1) * TQ, :],
                        in_ap=tq_t[q][:],
                        idxs_ap=idx_t[:, c0:c0 + IDXC],
                        num_idxs=CALL_IDXS,
                        num_idxs_reg=creg,
                        elem_size=de,
                        single_packet=False,
                        queue_num=q,
                    )
                a_g = agpool.tile([P, TILES * SPAN], bf16)
                nc.sync.dma_start(
                    out=a_g[:],
                    in_=aslab_in[:, g * TILES * SPAN:(g + 1) * TILES * SPAN])
                psum = pp.tile([SPAN, d], f32, space="PSUM")
                nc.tensor.matmul(
                    psum[:],
                    lhsT=xtg_t[:, g * SPAN:(g + 1) * SPAN],
                    rhs=waug_t[:],
                    start=True, stop=False,
                )
                for t in range(TILES):
                    nc.tensor.matmul(
                        psum[:], lhsT=a_g[:, t * SPAN:(t + 1) * SPAN],
                        rhs=msg[:, t, 0:d],
                        start=False, stop=(t == TILES - 1),
                    )
                ev = evp.tile([SPAN, d], f32)
                nc.vector.tensor_copy(out=ev[:], in_=psum[:])
                nc.sync.dma_start(out=stage[g * SPAN:(g + 1) * SPAN, :], in_=ev[:])
    nc.compile()
    return nc


def kernel(node_states, edge_weight, W, b, src_index, dst_index):
    node_states = np.asarray(node_states, dtype=np.float32)
    edge_weight = np.asarray(edge_weight, dtype=np.float32)
    W = np.asarray(W, dtype=np.float32)
    b = np.asarray(b, dtype=np.float32)
    src_index = np.asarray(src_index).astype(np.int64)
    dst_index = np.asarray(dst_index).astype(np.int64)

    N, d = node_states.shape
    E = src_index.shape[0]
    qrows = (N + 3) // 4
    assert qrows <= 32767, "int16 quarter index limit"

    order = np.argsort(dst_index, kind="stable")
    src_s = src_index[order]
    dst_s = dst_index[order]
    w_s = edge_weight[order]

    # core boundaries: ~equal edge counts, snapped to node boundaries
    node_bounds = [0]
    for k in range(1, N_CORES):
        nb = int(dst_s[min(k * E // N_CORES, E - 1)])
        node_bounds.append(max(nb, node_bounds[-1]))
    node_bounds.append(N)

    plans = []
    for k in range(N_CORES):
        n0, n1 = node_bounds[k], node_bounds[k + 1]
        e0 = np.searchsorted(dst_s, n0, "left")
        e1 = np.searchsorted(dst_s, n1, "left")
        plans.append((n0, n1, src_s[e0:e1], dst_s[e0:e1], w_s[e0:e1]))

    groups_per_core = [
        _plan_core(ss, ds, ws, n0, n1, qrows)
        for (n0, n1, ss, ds, ws) in plans
    ]
    G = max(len(g) for g in groups_per_core)

    # device program
    nc = _build_program(G, qrows, d)

    # table quarters (shared across cores): bf16 rows padded to 256B
    tpad = np.zeros((qrows * 4, 2 * d), BF16)
    tpad[:N, :d] = node_states.astype(BF16)
    quarters = {f"tq{q}": tpad[q * qrows:(q + 1) * qrows] for q in range(4)}
    waug = np.concatenate([W, b[None, :]], axis=0).astype(BF16)

    in_maps = []
    for k in range(N_CORES):
        n0, n1, ss, ds, ws = plans[k]
        groups = groups_per_core[k]
        idx_rep, aslab, cnts = _pack_core(ss, ds, ws, groups, G, qrows)
        xtg = np.zeros((d + 1, G * SPAN), BF16)
        for g, (a, bb) in enumerate(groups):
            xtg[:d, g * SPAN:g * SPAN + (bb - a)] = node_states[a:bb].T.astype(BF16)
            xtg[d, g * SPAN:g * SPAN + (bb - a)] = 1.0
        in_maps.append({
            **quarters,
            "idxs": idx_rep, "aslab": aslab,
            "xtg": xtg, "waug": waug, "cnts": cnts,
        })

    _LAST_RUN["nc"] = nc
    _LAST_RUN["in_maps"] = in_maps
    res = run_bass_kernel_spmd(nc, in_maps, list(range(N_CORES)))

    out = np.zeros((N, d), np.float32)
    for k in range(N_CORES):
        stage = res.results[k]["stage"]
        for g, (a, bb) in enumerate(groups_per_core[k]):
            out[a:bb] = stage[g * SPAN:g * SPAN + (bb - a)]
    return out
